# revision 36
# baseline (speedup 1.0000x reference)
"""Multi-head attention (B=4, S=1024, D=1024, H=16) on 8 trn2 NeuronCores.

Sharding: core c = b*2 + g handles batch b and head-group g (8 heads = 512 of
the 1024 hidden dims): data-parallel over B, tensor-parallel over heads.

Key compaction (host): masked keys (~half, Bernoulli mask) contribute exactly
zero attention, so the host gathers the unmasked key/value rows per batch and
pads to KP=640 (pad columns get a -1e10 exp bias -> exact zeros). Falls back
to a KP=1024 module if a mask is ever denser.

Per-core device pipeline (transposed [feature, seq] layout; all matmuls bf16):
  - qT/kT = W[g-slice] @ X^T on PE, resident-X chunked accumulation; q/k
    biases via K=1 ones-matmuls; 1/sqrt(hd) folded into qT's PSUM copy-out.
    q/k stored as 4 head-pair tiles [128, S] (head 2i rows 0-63, 2i+1 64-127).
  - attention is pipelined per head-pair at k-tile granularity: scores^T for
    pair p+1 (K=64 matmuls, two heads on disjoint PE row groups) interleave
    with ctx^T matmuls for pair p, keeping TensorE and ScalarE (exp) busy
    simultaneously.
  - exp on ScalarE over [128, 2x512] PSUM tiles, attention mask applied as a
    per-partition additive bias.
  - ctx^T and the softmax denominator Z come from one accumulated matmul per
    (head, q-chunk) with [v | ones] as the stationary operand (M=65).
  - Z is broadcast across partitions on GpSimd, 1/Z via a fast Newton DVE op;
    ctx^T normalized on-chip. UNNORMALIZED exp(scores^T) (bf16) and Z (fp32)
    stream to DRAM; the host divides + transposes while assembling attn.
  - output projection is fused into the attention tail per pair-couple and
    accumulated on VectorE into an SBUF tile; v/o biases are added exactly on
    the host (out += bv @ Wo.T + bo).

Measured: ~170 us HW exec across 8 cores, rel err ~6e-3 vs the fp32 reference
(bf16 datapath; set PREC="f32r" for ~4e-4 at ~2.3x the runtime).
"""

import numpy as np

B, S, D, H = 4, 1024, 1024, 16
HD = D // H          # 64 head dim
G = 2                # head groups -> 8 cores = B * G
HG = H // G          # 8 heads per core
R = HG * HD          # 512 feature rows per core
NC = 128             # partitions
DC = D // NC         # 8 d-chunks
KT = S // NC         # 8 key tiles
QCH = 512            # matmul moving free dim (PSUM bank)
NQC = S // QCH       # 2 query chunks
RC = R // NC         # 4 head-pair chunks
NEG = -1e10
PREC = "bf16"        # matmul datapath dtype: "bf16" or "f32r"
KP_DEFAULT = 640     # compacted+padded key count (mask keeps ~512 of 1024)

_CACHE = {}


def _build(KP):
    KTK = KP // NC           # key tiles after compaction
    kch = [(i * QCH, min(QCH, KP - i * QCH)) for i in range((KP + QCH - 1) // QCH)]
    import concourse.mybir as mybir
    from concourse import bacc
    from concourse.tile import TileContext
    from contextlib import ExitStack

    f32 = mybir.dt.float32
    f32r = mybir.dt.float32r
    mmd = mybir.dt.bfloat16 if PREC == "bf16" else f32r
    Exp = mybir.ActivationFunctionType.Exp
    Copy = mybir.ActivationFunctionType.Copy
    mult = mybir.AluOpType.mult

    nc = bacc.Bacc(None, target_bir_lowering=False)

    xqT = nc.dram_tensor("xqT", [D, S], mmd, kind="ExternalInput")
    xkT = nc.dram_tensor("xkT", [D, KP], mmd, kind="ExternalInput")
    xvT = nc.dram_tensor("xvT", [D, KP], mmd, kind="ExternalInput")
    wqT = nc.dram_tensor("wqT", [D, R], mmd, kind="ExternalInput")
    wkT = nc.dram_tensor("wkT", [D, R], mmd, kind="ExternalInput")
    wvT = nc.dram_tensor("wvT", [D, R], mmd, kind="ExternalInput")
    woT = nc.dram_tensor("woT", [R, D], mmd, kind="ExternalInput")
    maskb = nc.dram_tensor("maskb", [1, KP], f32, kind="ExternalInput")
    onesd = nc.dram_tensor("onesd", [1, S], mmd, kind="ExternalInput")
    bqd = nc.dram_tensor("bqd", [1, R], mmd, kind="ExternalInput")
    bkd = nc.dram_tensor("bkd", [1, R], mmd, kind="ExternalInput")
    # unnormalized exp(scores^T) per head [k, q], and Z per head [q]
    eout = nc.dram_tensor("eout", [HG, KP, S], mmd, kind="ExternalOutput")
    zout = nc.dram_tensor("zout", [HG, S], f32, kind="ExternalOutput")
    outp = nc.dram_tensor("outp", [S, D], f32, kind="ExternalOutput")

    with TileContext(nc) as tc, ExitStack() as top:
        persist = top.enter_context(tc.tile_pool(name="persist", bufs=1))
        work = top.enter_context(tc.tile_pool(name="work", bufs=2))

        # ---- constants / biases ----
        ones_sb = persist.tile([1, QCH], mmd, tag="ones")
        nc.gpsimd.dma_start(ones_sb[:], onesd[0:1, 0:QCH])
        maskb_sb = persist.tile([NC, KTK], f32, tag="maskb")
        nc.gpsimd.dma_start(
            maskb_sb[:], maskb[0:1, :].rearrange("a (t p) -> (a p) t", p=NC)
        )
        bq_sb = persist.tile([1, R], mmd, tag="bq")
        nc.gpsimd.dma_start(bq_sb[:], bqd[:])
        bk_sb = persist.tile([1, R], mmd, tag="bk")
        nc.gpsimd.dma_start(bk_sb[:], bkd[:])

        # ---- persistent activations ----
        wo_sb = [persist.tile([NC, D], mmd, tag=f"wo{p}", name=f"wo{p}") for p in range(RC)]
        for p in range(RC):
            nc.gpsimd.dma_start(wo_sb[p][:], woT[p * NC : (p + 1) * NC, :])
        out_acc = persist.tile([NC, S // NC, D], f32, tag="oacc")
        qTp = [persist.tile([NC, S], mmd, tag=f"qTp{p}", name=f"qTp{p}") for p in range(RC)]
        kTp = [persist.tile([NC, KP], mmd, tag=f"kTp{p}", name=f"kTp{p}") for p in range(RC)]
        v_aug = [persist.tile([NC, HG, HD + 1], mmd, tag=f"vau{t}", name=f"vau{t}") for t in range(KTK)]
        ctxT = [persist.tile([NC, S], mmd, tag=f"ctxT{p}", name=f"ctxT{p}") for p in range(RC)]

        ones3d = onesd[0:1, 0 : NC * HG].rearrange("a (p t o) -> (a p) t o", p=NC, o=1)
        for t in range(KTK):
            nc.gpsimd.dma_start(v_aug[t][:, :, HD : HD + 1], ones3d)

        # ---- projections (rc-major, resident x/w, overlap-friendly) ----
        psT = top.enter_context(tc.tile_pool(name="psT", bufs=2, space="PSUM"))
        psCU = top.enter_context(tc.tile_pool(name="psCU", bufs=4, space="PSUM"))
        stream = top.enter_context(
            tc.tile_pool(name="stream", bufs=2 if KP < S else 1)
        )
        stream1 = top.enter_context(tc.tile_pool(name="stream1", bufs=1))

        def big_ps():
            return psT.tile([NC, NQC, QCH], f32, tag="sT", name="sT")

        # v projection first (attention needs v_aug for every pair)
        xt = stream.tile([NC, DC, KP], mmd, tag="xv")
        wt = stream.tile([NC, DC, R], mmd, tag="w")
        for dc in range(DC):
            nc.sync.dma_start(wt[:, dc, :], wvT[dc * NC : (dc + 1) * NC, :])
            nc.sync.dma_start(xt[:, dc, :], xvT[dc * NC : (dc + 1) * NC, :])
        for st in range(KTK):
            ps = big_ps()[:, 0, :]
            for dc in range(DC):
                nc.tensor.matmul(
                    ps[:],
                    xt[:, dc, st * NC : (st + 1) * NC],
                    wt[:, dc, :],
                    start=(dc == 0),
                    stop=(dc == DC - 1),
                )
            nc.vector.tensor_copy(
                v_aug[st][:, :, 0:HD], ps[:].rearrange("p (h e) -> p h e", e=HD)
            )

        for which, xd, wd, b_sb in (
            ("k", xkT, wkT, bk_sb),
            ("q", xqT, wqT, bq_sb),
        ):
            SW = KP if which == "k" else S
            chunks = kch if which == "k" else [(i * QCH, QCH) for i in range(NQC)]
            xt = (stream if which == "k" else stream1).tile([NC, DC, SW], mmd, tag="xv" if which == "k" else "x")
            wt = stream.tile([NC, DC, R], mmd, tag="w")
            for dc in range(DC):
                nc.sync.dma_start(wt[:, dc, :], wd[dc * NC : (dc + 1) * NC, :])
                nc.sync.dma_start(xt[:, dc, :], xd[dc * NC : (dc + 1) * NC, :])
            for rc in range(RC):
                for off, size in chunks:
                    ps = big_ps()[:, 0, :size]
                    for dc in range(DC):
                        nc.tensor.matmul(
                            ps[:],
                            wt[:, dc, rc * NC : (rc + 1) * NC],
                            xt[:, dc, off : off + size],
                            start=(dc == 0),
                            stop=False,
                        )
                    nc.tensor.matmul(
                        ps[:],
                        b_sb[0:1, rc * NC : (rc + 1) * NC],
                        ones_sb[0:1, 0:size],
                        start=False,
                        stop=True,
                    )
                    dst = (qTp if which == "q" else kTp)[rc][:, off : off + size]
                    if which == "q":
                        nc.vector.tensor_scalar_mul(
                            dst, ps[:], 1.0 / float(np.sqrt(HD))
                        )
                    else:
                        nc.vector.tensor_copy(dst, ps[:])

        # ---- attention: pair-pipelined at k-tile granularity ----
        # scores/exp for pair p+1 interleave with ctx matmuls for pair p so
        # ScalarE (exp) and TensorE stay simultaneously busy.
        def get_expT(p):
            j0 = work.tile([NC, KTK, S], mmd, tag="expT0", name="expT0")
            j1 = work.tile([NC, KTK, S], mmd, tag="expT1", name="expT1")
            return [j0, j1]

        def emit_scores(p, kt, expT):
            pst = [
                psT.tile([NC, NQC, QCH], f32, tag="sT", name="sT") for _ in range(2)
            ]
            for sc in range(NQC):
                for j in range(2):
                    rows = slice(j * HD, (j + 1) * HD)
                    nc.tensor.matmul(
                        pst[j][:, sc, :],
                        kTp[p][rows, kt * NC : (kt + 1) * NC],
                        qTp[p][rows, sc * QCH : (sc + 1) * QCH],
                        start=True,
                        stop=True,
                        tile_position=(j * HD, 0),
                    )
            for j in range(2):
                nc.scalar.activation(
                    expT[j][:, kt, :],
                    pst[j][:],
                    Exp,
                    bias=maskb_sb[:, kt : kt + 1],
                )

        def finish_pair(p, pcus, expT):
            for j in range(2):
                h = 2 * p + j
                zrow = work.tile([1, S], f32, tag="zrow")
                for sc in range(NQC):
                    nc.vector.tensor_copy(
                        zrow[0:1, sc * QCH : (sc + 1) * QCH],
                        pcus[2 * j + sc][HD : HD + 1, :],
                    )
                nc.gpsimd.dma_start(zout[h : h + 1, :], zrow[:])
                zb = work.tile([NC, S], f32, tag="zb")
                nc.gpsimd.partition_broadcast(zb[:], zrow[:])
                sbc = work.tile([NC, S], f32, tag="sbc")
                nc.vector.reciprocal_approx_fast(out=sbc[:], in_=zb[:])
                for sc in range(NQC):
                    nc.vector.tensor_tensor(
                        ctxT[p][j * HD : (j + 1) * HD, sc * QCH : (sc + 1) * QCH],
                        pcus[2 * j + sc][0:HD, :],
                        sbc[0:HD, sc * QCH : (sc + 1) * QCH],
                        mult,
                    )
                nc.sync.dma_start(
                    eout[h].rearrange("(t p) q -> p t q", p=NC), expT[j][:]
                )

        def emit_outproj_couple(cp, dma=False):
            pcs = (2 * cp, 2 * cp + 1)
            for qt in range(S // NC):
                for oc in range(NQC):
                    po = psCU.tile([NC, QCH], f32, tag="cu", name="po")
                    for k, pc in enumerate(pcs):
                        nc.tensor.matmul(
                            po[:],
                            ctxT[pc][:, qt * NC : (qt + 1) * NC],
                            wo_sb[pc][:, oc * QCH : (oc + 1) * QCH],
                            start=(k == 0),
                            stop=(k == 1),
                        )
                    dst = out_acc[:, qt, oc * QCH : (oc + 1) * QCH]
                    if cp == 0:
                        nc.vector.tensor_copy(dst, po[:])
                    else:
                        nc.vector.tensor_tensor(dst, dst, po[:], mybir.AluOpType.add)
                if dma:
                    nc.sync.dma_start(
                        outp[qt * NC : (qt + 1) * NC, :], out_acc[:, qt, :]
                    )

        cur = get_expT(0)
        for kt in range(KTK):
            emit_scores(0, kt, cur)
        for p in range(RC):
            nxt = get_expT(p + 1) if p + 1 < RC else None
            pcus = [
                psCU.tile([HD + 1, QCH], f32, tag="cu", name="cu") for _ in range(4)
            ]
            for kt in range(KTK):
                if nxt is not None:
                    emit_scores(p + 1, kt, nxt)
                for j in range(2):
                    for sc in range(NQC):
                        nc.tensor.matmul(
                            pcus[2 * j + sc][:],
                            v_aug[kt][:, 2 * p + j, :],
                            cur[j][:, kt, sc * QCH : (sc + 1) * QCH],
                            start=(kt == 0),
                            stop=(kt == KTK - 1),
                        )
            finish_pair(p, pcus, cur)
            cur = nxt
            if p == 1:
                emit_outproj_couple(0)
            if p == RC - 1:
                emit_outproj_couple(1, dma=True)

    nc.finalize()
    return nc


def _get_nc(KP):
    key = f"nc{KP}"
    if key not in _CACHE:
        _CACHE[key] = _build(KP)
    return _CACHE[key]


def kernel(query, key, value, mask, Wq, bq, Wk, bk, Wv, bv, Wo, bo, _trace=False):
    from concourse.bass_utils import run_bass_kernel_spmd
    import ml_dtypes

    f = np.float32
    md = ml_dtypes.bfloat16 if PREC == "bf16" else np.float32

    def cast(a):
        return np.ascontiguousarray(np.asarray(a).astype(md))

    query = np.asarray(query, f)
    key = np.asarray(key, f)
    value = np.asarray(value, f)
    mask = np.asarray(mask)
    Wq, bq = np.asarray(Wq, f), np.asarray(bq, f)
    Wk, bk = np.asarray(Wk, f), np.asarray(bk, f)
    Wv, bv = np.asarray(Wv, f), np.asarray(bv, f)
    Wo, bo = np.asarray(Wo, f), np.asarray(bo, f)

    # compact unmasked keys per batch (masked keys contribute exactly 0)
    idxs = [np.nonzero(mask[b, 0, 0, :] != 0)[0] for b in range(B)]
    nmax = max(len(ix) for ix in idxs)
    KP = KP_DEFAULT if nmax <= KP_DEFAULT else S
    nc = _get_nc(KP)

    in_maps = []
    for c in range(B * G):
        b, g = divmod(c, G)
        rs = slice(g * R, (g + 1) * R)
        ix = idxs[b]
        n = len(ix)
        xk_c = np.zeros((KP, D), f)
        xk_c[:n] = key[b][ix]
        xv_c = np.zeros((KP, D), f)
        xv_c[:n] = value[b][ix]
        mb = np.full((1, KP), NEG, f)
        mb[0, :n] = 0.0
        in_maps.append(
            {
                "xqT": cast(query[b].T),
                "xkT": cast(xk_c.T),
                "xvT": cast(xv_c.T),
                "wqT": cast(Wq[rs, :].T),
                "wkT": cast(Wk[rs, :].T),
                "wvT": cast(Wv[rs, :].T),
                "woT": cast(Wo[:, rs].T),
                "maskb": np.ascontiguousarray(mb),
                "onesd": np.ones((1, S), md),
                "bqd": cast(bq[rs][None, :]),
                "bkd": cast(bk[rs][None, :]),
            }
        )

    res = run_bass_kernel_spmd(nc, in_maps, core_ids=list(range(B * G)), trace=_trace)
    _CACHE["last_results"] = res

    out = np.empty((B, S, D), f)
    attn = np.zeros((B, H, S, S), f)
    for c in range(B * G):
        b, g = divmod(c, G)
        r = res.results[c]
        if g == 0:
            out[b] = r["outp"]
        else:
            out[b] += r["outp"]
            out[b] += bv @ Wo.T + bo
        e = r["eout"]  # [HG, KP(k'), S(q)] unnormalized exp, bf16
        z = r["zout"]  # [HG, S(q)] fp32
        ix = idxs[b]
        n = len(ix)
        for h in range(HG):
            a = e[h][:n].astype(f)
            a /= z[h][None, :]
            attn[b, g * HG + h][:, ix] = a.T
    return out, attn


# revision 37
# speedup vs baseline: 1.0187x; 1.0187x over previous
"""Multi-head attention (B=4, S=1024, D=1024, H=16) on 8 trn2 NeuronCores.

Sharding: core c = b*2 + g handles batch b and head-group g (8 heads = 512 of
the 1024 hidden dims): data-parallel over B, tensor-parallel over heads.

Key compaction (host): masked keys (~half, Bernoulli mask) contribute exactly
zero attention, so the host gathers the unmasked key/value rows per batch and
pads to KP=640 (pad columns get a -1e10 exp bias -> exact zeros). Falls back
to a KP=1024 module if a mask is ever denser.

Per-core device pipeline (transposed [feature, seq] layout; all matmuls bf16):
  - qT/kT = W[g-slice] @ X^T on PE, resident-X chunked accumulation; q/k
    biases via K=1 ones-matmuls; 1/sqrt(hd) folded into qT's PSUM copy-out.
    q/k stored as 4 head-pair tiles [128, S] (head 2i rows 0-63, 2i+1 64-127).
  - attention is pipelined per head-pair at k-tile granularity: scores^T for
    pair p+1 (K=64 matmuls, two heads on disjoint PE row groups) interleave
    with ctx^T matmuls for pair p, keeping TensorE and ScalarE (exp) busy
    simultaneously.
  - exp on ScalarE over [128, 2x512] PSUM tiles, attention mask applied as a
    per-partition additive bias.
  - ctx^T and the softmax denominator Z come from one accumulated matmul per
    (head, q-chunk) with [v | ones] as the stationary operand (M=65).
  - Z is broadcast across partitions on GpSimd, 1/Z via a fast Newton DVE op;
    ctx^T normalized on-chip. UNNORMALIZED exp(scores^T) (bf16) and Z (fp32)
    stream to DRAM; the host divides + transposes while assembling attn.
  - output projection is fused into the attention tail per pair-couple and
    accumulated on VectorE into an SBUF tile; v/o biases are added exactly on
    the host (out += bv @ Wo.T + bo).

Measured: ~170 us HW exec across 8 cores, rel err ~6e-3 vs the fp32 reference
(bf16 datapath; set PREC="f32r" for ~4e-4 at ~2.3x the runtime).
"""

import numpy as np

B, S, D, H = 4, 1024, 1024, 16
HD = D // H          # 64 head dim
G = 2                # head groups -> 8 cores = B * G
HG = H // G          # 8 heads per core
R = HG * HD          # 512 feature rows per core
NC = 128             # partitions
DC = D // NC         # 8 d-chunks
KT = S // NC         # 8 key tiles
QCH = 512            # matmul moving free dim (PSUM bank)
NQC = S // QCH       # 2 query chunks
RC = R // NC         # 4 head-pair chunks
NEG = -1e10
PREC = "bf16"        # matmul datapath dtype: "bf16" or "f32r"
KP_DEFAULT = 640     # compacted+padded key count (mask keeps ~512 of 1024)

_CACHE = {}


def _build(KP):
    KTK = KP // NC           # key tiles after compaction
    kch = [(i * QCH, min(QCH, KP - i * QCH)) for i in range((KP + QCH - 1) // QCH)]
    import concourse.mybir as mybir
    from concourse import bacc
    from concourse.tile import TileContext
    from contextlib import ExitStack

    f32 = mybir.dt.float32
    f32r = mybir.dt.float32r
    mmd = mybir.dt.bfloat16 if PREC == "bf16" else f32r
    Exp = mybir.ActivationFunctionType.Exp
    Copy = mybir.ActivationFunctionType.Copy
    mult = mybir.AluOpType.mult

    nc = bacc.Bacc(None, target_bir_lowering=False)

    xqT = nc.dram_tensor("xqT", [D, S], mmd, kind="ExternalInput")
    xkT = nc.dram_tensor("xkT", [D, KP], mmd, kind="ExternalInput")
    xvT = nc.dram_tensor("xvT", [D, KP], mmd, kind="ExternalInput")
    wqT = nc.dram_tensor("wqT", [D, R], mmd, kind="ExternalInput")
    wkT = nc.dram_tensor("wkT", [D, R], mmd, kind="ExternalInput")
    wvT = nc.dram_tensor("wvT", [D, R], mmd, kind="ExternalInput")
    woT = nc.dram_tensor("woT", [R, D], mmd, kind="ExternalInput")
    maskb = nc.dram_tensor("maskb", [1, KP], f32, kind="ExternalInput")
    onesd = nc.dram_tensor("onesd", [1, S], mmd, kind="ExternalInput")
    bqd = nc.dram_tensor("bqd", [1, R], mmd, kind="ExternalInput")
    bkd = nc.dram_tensor("bkd", [1, R], mmd, kind="ExternalInput")
    # unnormalized exp(scores^T) per head [k, q], and Z per head [q]
    eout = nc.dram_tensor("eout", [HG, KP, S], mmd, kind="ExternalOutput")
    zout = nc.dram_tensor("zout", [HG, S], f32, kind="ExternalOutput")
    outp = nc.dram_tensor("outp", [S, D], f32, kind="ExternalOutput")

    with TileContext(nc) as tc, ExitStack() as top:
        persist = top.enter_context(tc.tile_pool(name="persist", bufs=1))
        work = top.enter_context(tc.tile_pool(name="work", bufs=2))

        # ---- constants / biases ----
        ones_sb = persist.tile([1, QCH], mmd, tag="ones")
        nc.gpsimd.dma_start(ones_sb[:], onesd[0:1, 0:QCH])
        maskb_sb = persist.tile([NC, KTK], f32, tag="maskb")
        nc.gpsimd.dma_start(
            maskb_sb[:], maskb[0:1, :].rearrange("a (t p) -> (a p) t", p=NC)
        )
        bq_sb = persist.tile([1, R], mmd, tag="bq")
        nc.gpsimd.dma_start(bq_sb[:], bqd[:])
        bk_sb = persist.tile([1, R], mmd, tag="bk")
        nc.gpsimd.dma_start(bk_sb[:], bkd[:])

        # ---- persistent activations ----
        wo_sb = [persist.tile([NC, D], mmd, tag=f"wo{p}", name=f"wo{p}") for p in range(RC)]
        for p in range(RC):
            nc.gpsimd.dma_start(wo_sb[p][:], woT[p * NC : (p + 1) * NC, :])
        out_acc = persist.tile([NC, S // NC, D], f32, tag="oacc")
        qTp = [persist.tile([NC, S], mmd, tag=f"qTp{p}", name=f"qTp{p}") for p in range(RC)]
        kTp = [persist.tile([NC, KP], mmd, tag=f"kTp{p}", name=f"kTp{p}") for p in range(RC)]
        v_aug = [persist.tile([NC, HG, HD + 1], mmd, tag=f"vau{t}", name=f"vau{t}") for t in range(KTK)]
        ctxT = [persist.tile([NC, S], mmd, tag=f"ctxT{p}", name=f"ctxT{p}") for p in range(RC)]

        ones3d = onesd[0:1, 0 : NC * HG].rearrange("a (p t o) -> (a p) t o", p=NC, o=1)
        for t in range(KTK):
            nc.gpsimd.dma_start(v_aug[t][:, :, HD : HD + 1], ones3d)

        # ---- projections (rc-major, resident x/w, overlap-friendly) ----
        psT = top.enter_context(tc.tile_pool(name="psT", bufs=2, space="PSUM"))
        psCU = top.enter_context(tc.tile_pool(name="psCU", bufs=4, space="PSUM"))
        stream = top.enter_context(
            tc.tile_pool(name="stream", bufs=2 if KP < S else 1)
        )
        stream1 = top.enter_context(tc.tile_pool(name="stream1", bufs=1))

        def big_ps():
            return psT.tile([NC, NQC, QCH], f32, tag="sT", name="sT")

        # v projection first (attention needs v_aug for every pair)
        xt = stream.tile([NC, DC, KP], mmd, tag="xv")
        wt = stream.tile([NC, DC, R], mmd, tag="w")
        for dc in range(DC):
            nc.sync.dma_start(wt[:, dc, :], wvT[dc * NC : (dc + 1) * NC, :])
            nc.sync.dma_start(xt[:, dc, :], xvT[dc * NC : (dc + 1) * NC, :])
        for st in range(KTK):
            ps = big_ps()[:, 0, :]
            for dc in range(DC):
                nc.tensor.matmul(
                    ps[:],
                    xt[:, dc, st * NC : (st + 1) * NC],
                    wt[:, dc, :],
                    start=(dc == 0),
                    stop=(dc == DC - 1),
                )
            nc.vector.tensor_copy(
                v_aug[st][:, :, 0:HD], ps[:].rearrange("p (h e) -> p h e", e=HD)
            )

        for which, xd, wd, b_sb in (
            ("k", xkT, wkT, bk_sb),
            ("q", xqT, wqT, bq_sb),
        ):
            SW = KP if which == "k" else S
            chunks = kch if which == "k" else [(i * QCH, QCH) for i in range(NQC)]
            xt = (stream if which == "k" else stream1).tile([NC, DC, SW], mmd, tag="xv" if which == "k" else "x")
            wt = stream.tile([NC, DC, R], mmd, tag="w")
            for dc in range(DC):
                nc.sync.dma_start(wt[:, dc, :], wd[dc * NC : (dc + 1) * NC, :])
                nc.sync.dma_start(xt[:, dc, :], xd[dc * NC : (dc + 1) * NC, :])
            for rc in range(RC):
                for off, size in chunks:
                    ps = big_ps()[:, 0, :size]
                    for dc in range(DC):
                        nc.tensor.matmul(
                            ps[:],
                            wt[:, dc, rc * NC : (rc + 1) * NC],
                            xt[:, dc, off : off + size],
                            start=(dc == 0),
                            stop=False,
                        )
                    nc.tensor.matmul(
                        ps[:],
                        b_sb[0:1, rc * NC : (rc + 1) * NC],
                        ones_sb[0:1, 0:size],
                        start=False,
                        stop=True,
                    )
                    dst = (qTp if which == "q" else kTp)[rc][:, off : off + size]
                    if which == "q":
                        nc.vector.tensor_scalar_mul(
                            dst, ps[:], 1.0 / float(np.sqrt(HD))
                        )
                    else:
                        nc.vector.tensor_copy(dst, ps[:])

        # ---- attention: pair-pipelined at k-tile granularity ----
        # scores/exp for pair p+1 interleave with ctx matmuls for pair p so
        # ScalarE (exp) and TensorE stay simultaneously busy.
        def get_expT(p):
            j0 = work.tile([NC, KTK, S], mmd, tag="expT0", name="expT0")
            j1 = work.tile([NC, KTK, S], mmd, tag="expT1", name="expT1")
            return [j0, j1]

        def emit_scores(p, kt, expT):
            pst = [
                psT.tile([NC, NQC, QCH], f32, tag="sT", name="sT") for _ in range(2)
            ]
            for sc in range(NQC):
                for j in range(2):
                    rows = slice(j * HD, (j + 1) * HD)
                    nc.tensor.matmul(
                        pst[j][:, sc, :],
                        kTp[p][rows, kt * NC : (kt + 1) * NC],
                        qTp[p][rows, sc * QCH : (sc + 1) * QCH],
                        start=True,
                        stop=True,
                        tile_position=(j * HD, 0),
                    )
            for j in range(2):
                nc.scalar.activation(
                    expT[j][:, kt, :],
                    pst[j][:],
                    Exp,
                    bias=maskb_sb[:, kt : kt + 1],
                )

        def finish_pair(p, pcus, expT):
            for j in range(2):
                h = 2 * p + j
                zrow = work.tile([1, S], f32, tag="zrow")
                for sc in range(NQC):
                    nc.vector.tensor_copy(
                        zrow[0:1, sc * QCH : (sc + 1) * QCH],
                        pcus[2 * j + sc][HD : HD + 1, :],
                    )
                nc.gpsimd.dma_start(zout[h : h + 1, :], zrow[:])
                zb = work.tile([NC, S], f32, tag="zb")
                nc.gpsimd.partition_broadcast(zb[:], zrow[:])
                sbc = work.tile([NC, S], f32, tag="sbc")
                nc.vector.reciprocal_approx_fast(out=sbc[:], in_=zb[:])
                for sc in range(NQC):
                    nc.vector.tensor_tensor(
                        ctxT[p][j * HD : (j + 1) * HD, sc * QCH : (sc + 1) * QCH],
                        pcus[2 * j + sc][0:HD, :],
                        sbc[0:HD, sc * QCH : (sc + 1) * QCH],
                        mult,
                    )
                nc.sync.dma_start(
                    eout[h].rearrange("(t p) q -> p t q", p=NC), expT[j][:]
                )

        def emit_outproj_tasks(cp, tasks, dma=False):
            pcs = (2 * cp, 2 * cp + 1)
            for qt, oc in tasks:
                if True:
                    po = psT.tile([NC, NQC, QCH], f32, tag="sT", name="po")[:, 0, :]
                    for k, pc in enumerate(pcs):
                        nc.tensor.matmul(
                            po[:],
                            ctxT[pc][:, qt * NC : (qt + 1) * NC],
                            wo_sb[pc][:, oc * QCH : (oc + 1) * QCH],
                            start=(k == 0),
                            stop=(k == 1),
                        )
                    dst = out_acc[:, qt, oc * QCH : (oc + 1) * QCH]
                    if cp == 0:
                        nc.vector.tensor_copy(dst, po[:])
                    else:
                        nc.vector.tensor_tensor(dst, dst, po[:], mybir.AluOpType.add)
                    if dma and oc == NQC - 1:
                        nc.sync.dma_start(
                            outp[qt * NC : (qt + 1) * NC, :], out_acc[:, qt, :]
                        )

        cur = get_expT(0)
        for kt in range(KTK):
            emit_scores(0, kt, cur)
        for p in range(RC):
            nxt = get_expT(p + 1) if p + 1 < RC else None
            pcus = [
                psCU.tile([HD + 1, QCH], f32, tag="cu", name="cu") for _ in range(4)
            ]
            tasks = (
                [(qt, oc) for qt in range(S // NC) for oc in range(NQC)]
                if p == 2
                else []
            )
            per_kt = (len(tasks) + KTK - 1) // KTK
            for kt in range(KTK):
                if nxt is not None:
                    emit_scores(p + 1, kt, nxt)
                for j in range(2):
                    for sc in range(NQC):
                        nc.tensor.matmul(
                            pcus[2 * j + sc][:],
                            v_aug[kt][:, 2 * p + j, :],
                            cur[j][:, kt, sc * QCH : (sc + 1) * QCH],
                            start=(kt == 0),
                            stop=(kt == KTK - 1),
                        )
                if tasks:
                    emit_outproj_tasks(0, tasks[kt * per_kt : (kt + 1) * per_kt])
            finish_pair(p, pcus, cur)
            cur = nxt
            if p == RC - 1:
                emit_outproj_tasks(
                    1,
                    [(qt, oc) for qt in range(S // NC) for oc in range(NQC)],
                    dma=True,
                )

    nc.finalize()
    return nc


def _get_nc(KP):
    key = f"nc{KP}"
    if key not in _CACHE:
        _CACHE[key] = _build(KP)
    return _CACHE[key]


def kernel(query, key, value, mask, Wq, bq, Wk, bk, Wv, bv, Wo, bo, _trace=False):
    from concourse.bass_utils import run_bass_kernel_spmd
    import ml_dtypes

    f = np.float32
    md = ml_dtypes.bfloat16 if PREC == "bf16" else np.float32

    def cast(a):
        return np.ascontiguousarray(np.asarray(a).astype(md))

    query = np.asarray(query, f)
    key = np.asarray(key, f)
    value = np.asarray(value, f)
    mask = np.asarray(mask)
    Wq, bq = np.asarray(Wq, f), np.asarray(bq, f)
    Wk, bk = np.asarray(Wk, f), np.asarray(bk, f)
    Wv, bv = np.asarray(Wv, f), np.asarray(bv, f)
    Wo, bo = np.asarray(Wo, f), np.asarray(bo, f)

    # compact unmasked keys per batch (masked keys contribute exactly 0)
    idxs = [np.nonzero(mask[b, 0, 0, :] != 0)[0] for b in range(B)]
    nmax = max(len(ix) for ix in idxs)
    KP = KP_DEFAULT if nmax <= KP_DEFAULT else S
    nc = _get_nc(KP)

    in_maps = []
    for c in range(B * G):
        b, g = divmod(c, G)
        rs = slice(g * R, (g + 1) * R)
        ix = idxs[b]
        n = len(ix)
        xk_c = np.zeros((KP, D), f)
        xk_c[:n] = key[b][ix]
        xv_c = np.zeros((KP, D), f)
        xv_c[:n] = value[b][ix]
        mb = np.full((1, KP), NEG, f)
        mb[0, :n] = 0.0
        in_maps.append(
            {
                "xqT": cast(query[b].T),
                "xkT": cast(xk_c.T),
                "xvT": cast(xv_c.T),
                "wqT": cast(Wq[rs, :].T),
                "wkT": cast(Wk[rs, :].T),
                "wvT": cast(Wv[rs, :].T),
                "woT": cast(Wo[:, rs].T),
                "maskb": np.ascontiguousarray(mb),
                "onesd": np.ones((1, S), md),
                "bqd": cast(bq[rs][None, :]),
                "bkd": cast(bk[rs][None, :]),
            }
        )

    res = run_bass_kernel_spmd(nc, in_maps, core_ids=list(range(B * G)), trace=_trace)
    _CACHE["last_results"] = res

    out = np.empty((B, S, D), f)
    attn = np.zeros((B, H, S, S), f)
    for c in range(B * G):
        b, g = divmod(c, G)
        r = res.results[c]
        if g == 0:
            out[b] = r["outp"]
        else:
            out[b] += r["outp"]
            out[b] += bv @ Wo.T + bo
        e = r["eout"]  # [HG, KP(k'), S(q)] unnormalized exp, bf16
        z = r["zout"]  # [HG, S(q)] fp32
        ix = idxs[b]
        n = len(ix)
        for h in range(HG):
            a = e[h][:n].astype(f)
            a /= z[h][None, :]
            attn[b, g * HG + h][:, ix] = a.T
    return out, attn


# revision 38
# speedup vs baseline: 1.0411x; 1.0220x over previous
"""Multi-head attention (B=4, S=1024, D=1024, H=16) on 8 trn2 NeuronCores.

Sharding: core c = b*2 + g handles batch b and head-group g (8 heads = 512 of
the 1024 hidden dims): data-parallel over B, tensor-parallel over heads.

Key compaction (host): masked keys (~half, Bernoulli mask) contribute exactly
zero attention, so the host gathers the unmasked key/value rows per batch and
pads to KP=640 (pad columns get a -1e10 exp bias -> exact zeros). Falls back
to a KP=1024 module if a mask is ever denser.

Per-core device pipeline (transposed [feature, seq] layout; all matmuls bf16):
  - qT/kT = W[g-slice] @ X^T on PE, resident-X chunked accumulation; q/k
    biases via K=1 ones-matmuls; 1/sqrt(hd) folded into qT's PSUM copy-out.
    q/k stored as 4 head-pair tiles [128, S] (head 2i rows 0-63, 2i+1 64-127).
  - attention is pipelined per head-pair at k-tile granularity: scores^T for
    pair p+1 (K=64 matmuls, two heads on disjoint PE row groups) interleave
    with ctx^T matmuls for pair p, keeping TensorE and ScalarE (exp) busy
    simultaneously.
  - exp on ScalarE over [128, 2x512] PSUM tiles, attention mask applied as a
    per-partition additive bias.
  - ctx^T and the softmax denominator Z come from one accumulated matmul per
    (head, q-chunk) with [v | ones] as the stationary operand (M=65).
  - Z is broadcast across partitions on GpSimd, 1/Z via a fast Newton DVE op;
    ctx^T normalized on-chip. UNNORMALIZED exp(scores^T) (bf16) and Z (fp32)
    stream to DRAM; the host divides + transposes while assembling attn.
  - output projection is fused into the attention tail per pair-couple and
    accumulated on VectorE into an SBUF tile; v/o biases are added exactly on
    the host (out += bv @ Wo.T + bo).

Measured: ~170 us HW exec across 8 cores, rel err ~6e-3 vs the fp32 reference
(bf16 datapath; set PREC="f32r" for ~4e-4 at ~2.3x the runtime).
"""

import numpy as np

B, S, D, H = 4, 1024, 1024, 16
HD = D // H          # 64 head dim
G = 2                # head groups -> 8 cores = B * G
HG = H // G          # 8 heads per core
R = HG * HD          # 512 feature rows per core
NC = 128             # partitions
DC = D // NC         # 8 d-chunks
KT = S // NC         # 8 key tiles
QCH = 512            # matmul moving free dim (PSUM bank)
NQC = S // QCH       # 2 query chunks
RC = R // NC         # 4 head-pair chunks
NEG = -1e10
PREC = "bf16"        # matmul datapath dtype: "bf16" or "f32r"
KP_DEFAULT = 640     # compacted+padded key count (mask keeps ~512 of 1024)

_CACHE = {}


def _build(KP):
    KTK = KP // NC           # key tiles after compaction
    kch = [(i * QCH, min(QCH, KP - i * QCH)) for i in range((KP + QCH - 1) // QCH)]
    import concourse.mybir as mybir
    from concourse import bacc
    from concourse.tile import TileContext
    from contextlib import ExitStack

    f32 = mybir.dt.float32
    f32r = mybir.dt.float32r
    mmd = mybir.dt.bfloat16 if PREC == "bf16" else f32r
    Exp = mybir.ActivationFunctionType.Exp
    Copy = mybir.ActivationFunctionType.Copy
    mult = mybir.AluOpType.mult

    nc = bacc.Bacc(None, target_bir_lowering=False)

    xqT = nc.dram_tensor("xqT", [D, S], mmd, kind="ExternalInput")
    xkT = nc.dram_tensor("xkT", [D, KP], mmd, kind="ExternalInput")
    xvT = nc.dram_tensor("xvT", [D, KP], mmd, kind="ExternalInput")
    wqT = nc.dram_tensor("wqT", [D, R], mmd, kind="ExternalInput")
    wkT = nc.dram_tensor("wkT", [D, R], mmd, kind="ExternalInput")
    wvT = nc.dram_tensor("wvT", [D, R], mmd, kind="ExternalInput")
    woT = nc.dram_tensor("woT", [R, D], mmd, kind="ExternalInput")
    maskb = nc.dram_tensor("maskb", [1, KP], f32, kind="ExternalInput")
    onesd = nc.dram_tensor("onesd", [1, S], mmd, kind="ExternalInput")
    bqd = nc.dram_tensor("bqd", [1, R], mmd, kind="ExternalInput")
    bkd = nc.dram_tensor("bkd", [1, R], mmd, kind="ExternalInput")
    # unnormalized exp(scores^T) per head [k, q], and Z per head [q]
    eout = nc.dram_tensor("eout", [HG, KP, S], mmd, kind="ExternalOutput")
    zout = nc.dram_tensor("zout", [HG, S], f32, kind="ExternalOutput")
    outp = nc.dram_tensor("outp", [S, D], f32, kind="ExternalOutput")

    with TileContext(nc) as tc, ExitStack() as top:
        persist = top.enter_context(tc.tile_pool(name="persist", bufs=1))
        work = top.enter_context(tc.tile_pool(name="work", bufs=2))

        # ---- constants / biases ----
        ones_sb = persist.tile([1, QCH], mmd, tag="ones")
        nc.gpsimd.dma_start(ones_sb[:], onesd[0:1, 0:QCH])
        maskb_sb = persist.tile([NC, KTK], f32, tag="maskb")
        nc.gpsimd.dma_start(
            maskb_sb[:], maskb[0:1, :].rearrange("a (t p) -> (a p) t", p=NC)
        )
        bq_sb = persist.tile([1, R], mmd, tag="bq")
        nc.gpsimd.dma_start(bq_sb[:], bqd[:])
        bk_sb = persist.tile([1, R], mmd, tag="bk")
        nc.gpsimd.dma_start(bk_sb[:], bkd[:])

        # ---- persistent activations ----
        wo_sb = [persist.tile([NC, D], mmd, tag=f"wo{p}", name=f"wo{p}") for p in range(RC)]
        for p in range(RC):
            nc.gpsimd.dma_start(wo_sb[p][:], woT[p * NC : (p + 1) * NC, :])
        out_acc = persist.tile([NC, S // NC, D], f32, tag="oacc")
        qTp = [persist.tile([NC, S], mmd, tag=f"qTp{p}", name=f"qTp{p}") for p in range(RC)]
        kTp = [persist.tile([NC, KP], mmd, tag=f"kTp{p}", name=f"kTp{p}") for p in range(RC)]
        v_aug = [persist.tile([NC, HG, HD + 1], mmd, tag=f"vau{t}", name=f"vau{t}") for t in range(KTK)]
        ctxT = [persist.tile([NC, S], mmd, tag=f"ctxT{p}", name=f"ctxT{p}") for p in range(RC)]

        ones3d = onesd[0:1, 0 : NC * HG].rearrange("a (p t o) -> (a p) t o", p=NC, o=1)
        for t in range(KTK):
            nc.gpsimd.dma_start(v_aug[t][:, :, HD : HD + 1], ones3d)

        # ---- projections (rc-major, resident x/w, overlap-friendly) ----
        psT = top.enter_context(tc.tile_pool(name="psT", bufs=2, space="PSUM"))
        psCU = top.enter_context(tc.tile_pool(name="psCU", bufs=4, space="PSUM"))
        stream = top.enter_context(
            tc.tile_pool(name="stream", bufs=2 if KP < S else 1)
        )
        stream1 = top.enter_context(tc.tile_pool(name="stream1", bufs=1))

        def big_ps():
            return psT.tile([NC, NQC, QCH], f32, tag="sT", name="sT")

        # v projection first (attention needs v_aug for every pair)
        xt = stream.tile([NC, DC, KP], mmd, tag="xv")
        wt = stream.tile([NC, DC, R], mmd, tag="w")
        for dc in range(DC):
            nc.sync.dma_start(wt[:, dc, :], wvT[dc * NC : (dc + 1) * NC, :])
            nc.sync.dma_start(xt[:, dc, :], xvT[dc * NC : (dc + 1) * NC, :])
        for st in range(KTK):
            ps = big_ps()[:, 0, :]
            for dc in range(DC):
                nc.tensor.matmul(
                    ps[:],
                    xt[:, dc, st * NC : (st + 1) * NC],
                    wt[:, dc, :],
                    start=(dc == 0),
                    stop=(dc == DC - 1),
                )
            nc.vector.tensor_copy(
                v_aug[st][:, :, 0:HD], ps[:].rearrange("p (h e) -> p h e", e=HD)
            )

        for which, xd, wd, b_sb in (
            ("k", xkT, wkT, bk_sb),
            ("q", xqT, wqT, bq_sb),
        ):
            SW = KP if which == "k" else S
            chunks = kch if which == "k" else [(i * QCH, QCH) for i in range(NQC)]
            xt = (stream if which == "k" else stream1).tile([NC, DC, SW], mmd, tag="xv" if which == "k" else "x")
            wt = stream.tile([NC, DC, R], mmd, tag="w")
            for dc in range(DC):
                nc.sync.dma_start(wt[:, dc, :], wd[dc * NC : (dc + 1) * NC, :])
                nc.sync.dma_start(xt[:, dc, :], xd[dc * NC : (dc + 1) * NC, :])
            for rc in range(RC):
                for off, size in chunks:
                    ps = big_ps()[:, 0, :size]
                    for dc in range(DC):
                        nc.tensor.matmul(
                            ps[:],
                            wt[:, dc, rc * NC : (rc + 1) * NC],
                            xt[:, dc, off : off + size],
                            start=(dc == 0),
                            stop=False,
                        )
                    nc.tensor.matmul(
                        ps[:],
                        b_sb[0:1, rc * NC : (rc + 1) * NC],
                        ones_sb[0:1, 0:size],
                        start=False,
                        stop=True,
                    )
                    dst = (qTp if which == "q" else kTp)[rc][:, off : off + size]
                    if which == "q":
                        nc.vector.tensor_scalar_mul(
                            dst, ps[:], 1.0 / float(np.sqrt(HD))
                        )
                    else:
                        nc.vector.tensor_copy(dst, ps[:])

        # ---- attention: pair-pipelined at k-tile granularity ----
        # scores/exp for pair p+1 interleave with ctx matmuls for pair p so
        # ScalarE (exp) and TensorE stay simultaneously busy.
        def get_expT(p):
            j0 = work.tile([NC, KTK, S], mmd, tag="expT0", name="expT0")
            j1 = work.tile([NC, KTK, S], mmd, tag="expT1", name="expT1")
            return [j0, j1]

        def emit_scores(p, kt, expT):
            pst = [
                psT.tile([NC, NQC, QCH], f32, tag="sT", name="sT") for _ in range(2)
            ]
            for sc in range(NQC):
                for j in range(2):
                    rows = slice(j * HD, (j + 1) * HD)
                    nc.tensor.matmul(
                        pst[j][:, sc, :],
                        kTp[p][rows, kt * NC : (kt + 1) * NC],
                        qTp[p][rows, sc * QCH : (sc + 1) * QCH],
                        start=True,
                        stop=True,
                        tile_position=(j * HD, 0),
                    )
            for j in range(2):
                nc.scalar.activation(
                    expT[j][:, kt, :],
                    pst[j][:],
                    Exp,
                    bias=maskb_sb[:, kt : kt + 1],
                )

        def finish_pair(p, pcus, expT):
            for j in range(2):
                h = 2 * p + j
                zrow = work.tile([1, S], f32, tag="zrow")
                for sc in range(NQC):
                    nc.vector.tensor_copy(
                        zrow[0:1, sc * QCH : (sc + 1) * QCH],
                        pcus[2 * j + sc][HD : HD + 1, :],
                    )
                nc.gpsimd.dma_start(zout[h : h + 1, :], zrow[:])
                zb = work.tile([NC, S], f32, tag="zb")
                nc.gpsimd.partition_broadcast(zb[:], zrow[:])
                sbc = work.tile([NC, S], f32, tag="sbc")
                nc.vector.reciprocal_approx_fast(out=sbc[:], in_=zb[:])
                for sc in range(NQC):
                    nc.vector.tensor_tensor(
                        ctxT[p][j * HD : (j + 1) * HD, sc * QCH : (sc + 1) * QCH],
                        pcus[2 * j + sc][0:HD, :],
                        sbc[0:HD, sc * QCH : (sc + 1) * QCH],
                        mult,
                    )
                nc.sync.dma_start(
                    eout[h].rearrange("(t p) q -> p t q", p=NC), expT[j][:]
                )

        def emit_outproj_couple(cp, dma=False):
            pcs = (2 * cp, 2 * cp + 1)
            for qt in range(S // NC):
                for oc in range(NQC):
                    po = psCU.tile([NC, QCH], f32, tag="cu", name="po")
                    for k, pc in enumerate(pcs):
                        nc.tensor.matmul(
                            po[:],
                            ctxT[pc][:, qt * NC : (qt + 1) * NC],
                            wo_sb[pc][:, oc * QCH : (oc + 1) * QCH],
                            start=(k == 0),
                            stop=(k == 1),
                        )
                    dst = out_acc[:, qt, oc * QCH : (oc + 1) * QCH]
                    if cp == 0:
                        nc.vector.tensor_copy(dst, po[:])
                    else:
                        nc.vector.tensor_tensor(dst, dst, po[:], mybir.AluOpType.add)
                if dma:
                    nc.sync.dma_start(
                        outp[qt * NC : (qt + 1) * NC, :], out_acc[:, qt, :]
                    )

        cur = get_expT(0)
        for kt in range(KTK):
            emit_scores(0, kt, cur)
        for p in range(RC):
            nxt = get_expT(p + 1) if p + 1 < RC else None
            pcus = [
                psCU.tile([HD + 1, QCH], f32, tag="cu", name="cu") for _ in range(4)
            ]
            for kt in range(KTK):
                if nxt is not None:
                    emit_scores(p + 1, kt, nxt)
                for j in range(2):
                    for sc in range(NQC):
                        nc.tensor.matmul(
                            pcus[2 * j + sc][:],
                            v_aug[kt][:, 2 * p + j, :],
                            cur[j][:, kt, sc * QCH : (sc + 1) * QCH],
                            start=(kt == 0),
                            stop=(kt == KTK - 1),
                        )
            finish_pair(p, pcus, cur)
            cur = nxt
            if p == 1:
                emit_outproj_couple(0)
            if p == RC - 1:
                emit_outproj_couple(1, dma=True)

    nc.finalize()
    return nc


def _get_nc(KP):
    key = f"nc{KP}"
    if key not in _CACHE:
        _CACHE[key] = _build(KP)
    return _CACHE[key]


def kernel(query, key, value, mask, Wq, bq, Wk, bk, Wv, bv, Wo, bo, _trace=False):
    from concourse.bass_utils import run_bass_kernel_spmd
    import ml_dtypes

    f = np.float32
    md = ml_dtypes.bfloat16 if PREC == "bf16" else np.float32

    def cast(a):
        return np.ascontiguousarray(np.asarray(a).astype(md))

    query = np.asarray(query, f)
    key = np.asarray(key, f)
    value = np.asarray(value, f)
    mask = np.asarray(mask)
    Wq, bq = np.asarray(Wq, f), np.asarray(bq, f)
    Wk, bk = np.asarray(Wk, f), np.asarray(bk, f)
    Wv, bv = np.asarray(Wv, f), np.asarray(bv, f)
    Wo, bo = np.asarray(Wo, f), np.asarray(bo, f)

    # compact unmasked keys per batch (masked keys contribute exactly 0)
    idxs = [np.nonzero(mask[b, 0, 0, :] != 0)[0] for b in range(B)]
    nmax = max(len(ix) for ix in idxs)
    KP = KP_DEFAULT if nmax <= KP_DEFAULT else S
    nc = _get_nc(KP)

    in_maps = []
    for c in range(B * G):
        b, g = divmod(c, G)
        rs = slice(g * R, (g + 1) * R)
        ix = idxs[b]
        n = len(ix)
        xk_c = np.zeros((KP, D), f)
        xk_c[:n] = key[b][ix]
        xv_c = np.zeros((KP, D), f)
        xv_c[:n] = value[b][ix]
        mb = np.full((1, KP), NEG, f)
        mb[0, :n] = 0.0
        in_maps.append(
            {
                "xqT": cast(query[b].T),
                "xkT": cast(xk_c.T),
                "xvT": cast(xv_c.T),
                "wqT": cast(Wq[rs, :].T),
                "wkT": cast(Wk[rs, :].T),
                "wvT": cast(Wv[rs, :].T),
                "woT": cast(Wo[:, rs].T),
                "maskb": np.ascontiguousarray(mb),
                "onesd": np.ones((1, S), md),
                "bqd": cast(bq[rs][None, :]),
                "bkd": cast(bk[rs][None, :]),
            }
        )

    res = run_bass_kernel_spmd(nc, in_maps, core_ids=list(range(B * G)), trace=_trace)
    _CACHE["last_results"] = res

    out = np.empty((B, S, D), f)
    attn = np.zeros((B, H, S, S), f)
    for c in range(B * G):
        b, g = divmod(c, G)
        r = res.results[c]
        if g == 0:
            out[b] = r["outp"]
        else:
            out[b] += r["outp"]
            out[b] += bv @ Wo.T + bo
        e = r["eout"]  # [HG, KP(k'), S(q)] unnormalized exp, bf16
        z = r["zout"]  # [HG, S(q)] fp32
        ix = idxs[b]
        n = len(ix)
        for h in range(HG):
            a = e[h][:n].astype(f)
            a /= z[h][None, :]
            attn[b, g * HG + h][:, ix] = a.T
    return out, attn
